# revision 7
# baseline (speedup 1.0000x reference)
"""BandSplit (BSRNN-style) Trainium2 kernel.

Reference computation (per batch sample, per band):
  h   = moveaxis(x[:, :, s:e, :, :], -1, 1).reshape(B, 4w, T)   # channels (r, c, f)
  hn  = (h - mu) * rsqrt(var + eps) * gamma + beta              # GroupNorm(1, ch) over (ch, T)
  y   = W_band @ hn + b_band                                    # [128, T]
  out = stack over bands -> [B, 128, 31, T]

Folded form used here (r_b = rsqrt(var+eps); mu, r_b are per band+sample):
  y = r_b * (Wg @ h) + (v + b_band - r_b*mu*u)
  Wg = W * gamma (per column), u = Wg @ 1, v = W @ beta
so the big matmul runs on RAW h (no normalization pass over the data) and the
normalization is applied as a per-band scalar scale + per-output-channel bias
to the matmul output. Wg/u/v are parameter-only values and are prepared on the
host together with the weight layout packing; everything that touches x (the
matmuls, the mean/variance statistics, normalization, bias) runs on device.

Sharding: data-parallel over batch B=8 across the 8 NeuronCores (sample b on
core b); parameters replicated. Inside a core, h is never materialized:
matmuls read the natively-laid-out staged x tiles with a stride-2 free-dim
access pattern selecting the real/imag plane. Per-band sums come from
one-hot-stationary matmuls (partition reduction on the PE); sums of squares
from scalar-engine Square+accumulate passes.
"""

import numpy as np

import concourse.bass as bass
import concourse.tile as tile
from concourse import bacc, mybir

F32 = mybir.dt.float32
F32R = mybir.dt.float32r
AFT = mybir.ActivationFunctionType
ALU = mybir.AluOpType

# ---------------------------------------------------------------- problem dims
WIDTHS = [25] * 10 + [50] * 12 + [100] * 8 + [399]
NBANDS = len(WIDTHS)          # 31
C_IN = 2
T = 512
OUT_CH = 128
EPS = 1e-5
F_TOT = 2049
N_CORES = 8
N_WTP_PIECES = 8


def _tables():
    """Staged-tile and weight-chunk tables.

    A staged tile holds SBUF partitions = (c, f) rows of one band (c-stacked
    when 2w <= 128), free dim = the interleaved (t, r) row of 1024 floats.
    Each staged tile feeds two weight chunks (one per r), each a contiguous
    range of the band's channels (channel index = r*2w + c*w + f).
    """
    starts = np.concatenate([[0], np.cumsum(WIDTHS)])
    ch_off = np.concatenate([[0], np.cumsum([4 * w for w in WIDTHS])])
    staged = []
    for i, w in enumerate(WIDTHS):
        s = int(starts[i])
        off = int(ch_off[i])
        if 2 * w <= 128:
            staged.append(dict(
                band=i, f0=s, nf=w, cs=(0, 1), K=2 * w,
                ch=[(off, off + 2 * w), (off + 2 * w, off + 4 * w)],
            ))
        elif w <= 128:
            for c in range(2):
                staged.append(dict(
                    band=i, f0=s, nf=w, cs=(c,), K=w,
                    ch=[(off + r * 2 * w + c * w, off + r * 2 * w + c * w + w)
                        for r in range(2)],
                ))
        else:
            fsubs = [(f0, min(f0 + 128, w)) for f0 in range(0, w, 128)]
            for c in range(2):
                for (f0, f1) in fsubs:
                    staged.append(dict(
                        band=i, f0=s + f0, nf=f1 - f0, cs=(c,), K=f1 - f0,
                        ch=[(off + r * 2 * w + c * w + f0,
                             off + r * 2 * w + c * w + f1) for r in range(2)],
                    ))
    slots = []
    for ti, st in enumerate(staged):
        for r in range(2):
            slots.append(dict(tile=ti, r=r, ch=st["ch"][r]))
    # S2 strip column per staged tile; per-band ranges padded to EVEN width
    # (f32r matmuls require an even moving free size)
    s2col = {}
    band_s2cols = [[] for _ in range(NBANDS)]
    col = 0
    for band in range(NBANDS):
        tis = [ti for ti, st in enumerate(staged) if st["band"] == band]
        for k, ti in enumerate(tis):
            s2col[ti] = col + k
        width = len(tis) + (len(tis) % 2)
        band_s2cols[band] = list(range(col, col + width))
        col += width
    return staged, slots, s2col, band_s2cols, int(col)


STAGED, SLOTS, S2COL, BAND_S2COLS, N_S2COLS = _tables()
N_SLOTS = len(SLOTS)  # 92


def _pack_params(W, gamma, beta, bb):
    """Host-side preparation of the parameter-only tensors.

    wtp: gamma-scaled W^T packed into per-matmul 128-row slots
    uvb: [128, 2, 31] with [:, 0, i] = u_i (row-sums of Wg per band) and
         [:, 1, i] = v_i + b_i  (W @ beta per band, plus the conv bias)
    """
    Wg = (W * gamma[None, :]).astype(np.float32)
    WgT = np.ascontiguousarray(Wg.T)
    wtp = np.zeros((128, N_SLOTS, 128), np.float32)
    for j, sl in enumerate(SLOTS):
        a, e = sl["ch"]
        wtp[: e - a, j, :] = WgT[a:e, :]
    ch_off = np.concatenate([[0], np.cumsum([4 * w for w in WIDTHS])]).astype(int)
    uvb = np.zeros((128, 2, NBANDS), np.float32)
    for i in range(NBANDS):
        a, e = int(ch_off[i]), int(ch_off[i + 1])
        uvb[:, 0, i] = Wg[:, a:e].sum(axis=1)
        uvb[:, 1, i] = W[:, a:e] @ beta[a:e] + bb[i]
    return wtp, uvb


def _build_nc():
    nc = bacc.Bacc("TRN2")

    x_d = nc.dram_tensor("xb", [C_IN, F_TOT, T, 2], F32, kind="ExternalInput")
    wtp_d = nc.dram_tensor("wtp", [128, N_SLOTS, 128], F32, kind="ExternalInput")
    uvb_d = nc.dram_tensor("uvb", [128, 2, NBANDS], F32, kind="ExternalInput")
    y_d = nc.dram_tensor("y", [OUT_CH, NBANDS, T], F32, kind="ExternalOutput")

    # DRAM scratch for cross-partition broadcasts (written then read back)
    cvec_d = nc.dram_tensor("cvec_scratch", [1, NBANDS], F32)
    rpack_d = nc.dram_tensor("rpack_scratch", [NBANDS, 2], F32)

    with tile.TileContext(nc) as tc:
        with tc.tile_pool(name="persist", bufs=1) as persist, \
             tc.tile_pool(name="stage", bufs=8) as stage, \
             tc.tile_pool(name="scratch", bufs=2) as scratchp, \
             tc.tile_pool(name="outp", bufs=4) as outp, \
             tc.tile_pool(name="small", bufs=1) as small, \
             tc.tile_pool(name="psmain", bufs=6, space="PSUM") as psmain, \
             tc.tile_pool(name="psaux", bufs=1, space="PSUM") as psaux:

            # ------------- parameters (act-queue DMAs, in pieces) ----------
            piece = (N_SLOTS + N_WTP_PIECES - 1) // N_WTP_PIECES
            wtps = []
            slot_tile = []  # slot -> (piece_idx, local_slot)
            for p in range(N_WTP_PIECES):
                j0 = p * piece
                j1 = min(j0 + piece, N_SLOTS)
                wt = persist.tile([128, j1 - j0, 128], F32R, name=f"wtp{p}")
                nc.scalar.dma_start(out=wt, in_=wtp_d.bitcast(F32R)[:, j0:j1, :])
                wtps.append(wt)
                for j in range(j0, j1):
                    slot_tile.append((p, j - j0))
            uvb = persist.tile([128, 2, NBANDS], F32)
            nc.scalar.dma_start(out=uvb, in_=uvb_d[:])

            # one-hot matrix, M=32 slices: ohm[p, c] = (c == 31);
            # band b stationary = ohm[0:K, 31-b : 63-b] -> out row b
            ohm32 = small.tile([128, 63], F32)
            nc.vector.memset(ohm32, 0.0)
            nc.vector.memset(ohm32[:, 31:32], 1.0)
            ohm = small.tile([128, 63], F32R)
            nc.vector.tensor_copy(out=ohm, in_=ohm32)

            # per-band 1/(ch*T) constants -> [31, 1] via DRAM bounce
            cvec = small.tile([1, NBANDS], F32)
            for i, w in enumerate(WIDTHS):
                nc.vector.memset(cvec[0:1, i:i + 1], 1.0 / (4 * w * T))
            nc.gpsimd.dma_start(out=cvec_d[:], in_=cvec)
            invn = small.tile([NBANDS, 1], F32)
            src = cvec_d[0:1, :]
            nc.gpsimd.dma_start(
                out=invn,
                in_=bass.AP(tensor=src.tensor, offset=src.offset,
                            ap=[[1, NBANDS], [1, 1]]),
            )

            strip = small.tile([128, N_S2COLS], F32)
            nc.vector.memset(strip, 0.0)

            # ---------------- main streaming loop ------------------------
            s1ps = psaux.tile([32, T], F32)       # per-band sums, row = band
            osb = persist.tile([128, NBANDS, T], F32)  # raw y = Wg @ h

            band_nmm = {}
            for st in STAGED:
                band_nmm[st["band"]] = band_nmm.get(st["band"], 0) + 2
            band_done = {b_: 0 for b_ in band_nmm}
            band_psum = {}

            n_staged = len(STAGED)
            s1_idx = 0
            for ti, st in enumerate(STAGED):
                K = st["K"]
                xt = stage.tile([128, 1024], F32R, tag="xt", name=f"xt{ti}")
                if len(st["cs"]) == 2:
                    in_ap = x_d.bitcast(F32R)[:, st["f0"]: st["f0"] + st["nf"], :, :]
                else:
                    c = st["cs"][0]
                    in_ap = x_d.bitcast(F32R)[c, st["f0"]: st["f0"] + st["nf"], :, :]
                nc.sync.dma_start(out=xt[0:K, :], in_=in_ap)
                xv = xt.rearrange("p (t r) -> p t r", r=2)

                band = st["band"]
                if band not in band_psum:
                    band_psum[band] = psmain.tile([128, T], F32, tag="acc",
                                                  name=f"acc{band}")

                # main matmuls (one per r-plane)
                for r in range(2):
                    pi, lj = slot_tile[2 * ti + r]
                    band_done[band] += 1
                    nc.tensor.matmul(
                        band_psum[band][:],
                        wtps[pi][0:K, lj, :],
                        xv[0:K, :, r],
                        start=(band_done[band] == 1),
                        stop=(band_done[band] == band_nmm[band]),
                    )

                # S1 matmuls (contiguous halves, one-hot stationary -> row=band)
                for h in range(2):
                    s1_idx += 1
                    nc.tensor.matmul(
                        s1ps[:],
                        ohm[0:K, 31 - band: 63 - band],
                        xt[0:K, h * T: (h + 1) * T],
                        start=(s1_idx == 1),
                        stop=(s1_idx == 2 * n_staged),
                    )

                # S2: square + accumulate on the scalar engine
                sq = scratchp.tile([128, 1024], F32, tag="sq", name=f"sq{ti}")
                col = S2COL[ti]
                nc.scalar.activation(
                    out=sq[0:K, :],
                    in_=xt.bitcast(F32)[0:K, :],
                    func=AFT.Square,
                    accum_out=strip[0:K, col: col + 1],
                )

                # raw eviction once the band's accumulation is complete
                if band_done[band] == band_nmm[band]:
                    acc = band_psum.pop(band)
                    nc.vector.tensor_copy(out=osb[:, band, :], in_=acc[:])

            # ---------------- statistics -> r, rmu ------------------------
            strip_r = small.tile([128, N_S2COLS], F32R)
            nc.vector.tensor_copy(out=strip_r, in_=strip)
            s2ps = psaux.tile([32, N_S2COLS], F32)
            for band in range(NBANDS):
                cols = BAND_S2COLS[band]
                c0, c1 = cols[0], cols[-1] + 1
                nc.tensor.matmul(
                    s2ps[:, c0:c1],
                    ohm[0:128, 31 - band: 63 - band],
                    strip_r[:, c0:c1],
                    start=(band == 0), stop=(band == NBANDS - 1),
                )

            s1red = small.tile([NBANDS, 1], F32)
            nc.vector.tensor_reduce(out=s1red, in_=s1ps[0:NBANDS, :],
                                    axis=mybir.AxisListType.X, op=ALU.add)
            s2red = small.tile([NBANDS, 1], F32)
            nc.vector.tensor_reduce(out=s2red, in_=s2ps[0:NBANDS, :],
                                    axis=mybir.AxisListType.X, op=ALU.add)

            mu = small.tile([NBANDS, 1], F32)
            nc.vector.tensor_mul(out=mu, in0=s1red, in1=invn)
            ex2 = small.tile([NBANDS, 1], F32)
            nc.vector.tensor_mul(out=ex2, in0=s2red, in1=invn)
            musq = small.tile([NBANDS, 1], F32)
            nc.vector.tensor_mul(out=musq, in0=mu, in1=mu)
            var = small.tile([NBANDS, 1], F32)
            nc.vector.tensor_tensor(out=var, in0=ex2, in1=musq, op=ALU.subtract)
            epst = small.tile([NBANDS, 1], F32)
            nc.vector.memset(epst, EPS)
            std = small.tile([NBANDS, 1], F32)
            nc.scalar.activation(out=std, in_=var, func=AFT.Sqrt, bias=epst[:, 0:1])
            rpack = small.tile([NBANDS, 2], F32)
            nc.vector.reciprocal(out=rpack[:, 0:1], in_=std)
            nc.vector.tensor_mul(out=rpack[:, 1:2], in0=rpack[:, 0:1], in1=mu)

            # broadcast r / rmu to all partitions via DRAM bounce
            nc.gpsimd.dma_start(out=rpack_d[:], in_=rpack)
            rb = small.tile([128, NBANDS], F32)
            rmub = small.tile([128, NBANDS], F32)
            src_r = rpack_d[:, 0:1]
            nc.gpsimd.dma_start(
                out=rb,
                in_=bass.AP(tensor=src_r.tensor, offset=src_r.offset,
                            ap=[[0, 128], [2, NBANDS]]),
            )
            src_m = rpack_d[:, 1:2]
            nc.gpsimd.dma_start(
                out=rmub,
                in_=bass.AP(tensor=src_m.tensor, offset=src_m.offset,
                            ap=[[0, 128], [2, NBANDS]]),
            )

            # bias vectors: BB = (v + b) - rmu * u
            t_ru = small.tile([128, NBANDS], F32)
            nc.vector.tensor_mul(out=t_ru, in0=rmub, in1=uvb[:, 0, :])
            bbv = small.tile([128, NBANDS], F32)
            nc.vector.tensor_tensor(out=bbv, in0=uvb[:, 1, :], in1=t_ru,
                                    op=ALU.subtract)

            # ---------------- finalize + store ----------------------------
            for band in range(NBANDS):
                ot = outp.tile([128, T], F32, tag="ot", name=f"ot{band}")
                nc.vector.tensor_scalar(
                    out=ot,
                    in0=osb[:, band, :],
                    scalar1=rb[:, band: band + 1],
                    scalar2=bbv[:, band: band + 1],
                    op0=ALU.mult,
                    op1=ALU.add,
                )
                nc.scalar.dma_start(out=y_d[:, band, :], in_=ot)

    nc.finalize()
    return nc


_NC_CACHE = None


def _get_nc():
    global _NC_CACHE
    if _NC_CACHE is None:
        _NC_CACHE = _build_nc()
    return _NC_CACHE


def kernel(x, gamma, beta, W, b):
    from concourse.bass_utils import run_bass_kernel_spmd

    x = np.asarray(x, dtype=np.float32)
    gamma = np.asarray(gamma, dtype=np.float32)
    beta = np.asarray(beta, dtype=np.float32)
    W = np.asarray(W, dtype=np.float32)
    b = np.asarray(b, dtype=np.float32)

    wtp, uvb = _pack_params(W, gamma, beta, b)
    nc = _get_nc()
    in_maps = [
        {"xb": np.ascontiguousarray(x[i]), "wtp": wtp, "uvb": uvb}
        for i in range(N_CORES)
    ]
    res = run_bass_kernel_spmd(nc, in_maps, list(range(N_CORES)))
    return np.stack([res.results[i]["y"] for i in range(N_CORES)], axis=0)


# revision 11
# speedup vs baseline: 1.9461x; 1.9461x over previous
"""BandSplit (BSRNN-style) Trainium2 kernel.

Reference computation (per batch sample, per band of width w, ch = 4w):
  h   = moveaxis(x[:, :, s:e, :, :], -1, 1).reshape(B, ch, T)   # channels (r, c, f)
  hn  = (h - mu) * rsqrt(var + eps) * gamma + beta              # GroupNorm(1, ch) over (ch, T)
  y   = W_band @ hn + b_band                                    # [128, T]
  out = stack over bands -> [B, 128, 31, T]

Folded form used here (r_b = rsqrt(var+eps); mu, r_b are per band+sample):
  y = r_b * (Wg @ h) + (v + b_band - r_b*mu*u)
  Wg = W * gamma (per column), u = Wg @ 1, v = W @ beta
so the big matmul runs on RAW h (no normalization pass over the data) and the
normalization is applied as a per-band scalar scale + per-output-channel bias
to the matmul output. Wg/u/v are parameter-only values and are prepared on the
host together with the weight layout packing; everything that touches x (the
matmuls, the mean/variance statistics, normalization, bias) runs on device.

Sharding: data-parallel over batch B=8 across the 8 NeuronCores (sample b on
core b); parameters replicated. Inside a core, h is never materialized:
matmuls read the natively-laid-out staged x tiles (partitions = (c, f) rows,
free = interleaved (t, r)) with a stride-2 free-dim access pattern selecting
the real/imag plane. Per-band sums come from one-hot-stationary matmuls
(partition reduction on the PE); sums of squares from scalar-engine
Square+accumulate passes. x is staged in multi-band "super tiles" so each
DMA moves >= ~1 MB (wide hardware-queue fan-out).
"""

import numpy as np

import concourse.bass as bass
import concourse.tile as tile
from concourse import bacc, mybir

F32 = mybir.dt.float32
F32R = mybir.dt.float32r
AFT = mybir.ActivationFunctionType
ALU = mybir.AluOpType

# ---------------------------------------------------------------- problem dims
WIDTHS = [25] * 10 + [50] * 12 + [100] * 8 + [399]
NBANDS = len(WIDTHS)          # 31
C_IN = 2
T = 512
OUT_CH = 128
EPS = 1e-5
F_TOT = 2049
N_CORES = 8
N_WTP_PIECES = 4
OUT_GROUPS = [(0, 8), (8, 16), (16, 24), (24, 31)]

_STARTS = np.concatenate([[0], np.cumsum(WIDTHS)]).astype(int)
_CHOFF = np.concatenate([[0], np.cumsum([4 * w for w in WIDTHS])]).astype(int)


def _tables():
    """Super-tile staging plan + weight-chunk slots + S2 strip columns.

    Each super tile is one DMA; its free dim indexes "subs". A sub is one
    former staged tile: partitions = (c, f) rows of one band (c-stacked when
    2w <= 128), free row = interleaved (t, r) of 1024 floats. Each sub feeds
    two weight-chunk slots (one per r-plane), each a contiguous channel range
    (channel index within band = r*2w + c*w + f).
    """
    supers = []  # dict: kind, bands/c info for DMA AP, subs: [(band, K, [(a,e),(a,e)])]

    def band_sub(i, w, off):
        # c-stacked sub for a 2w<=128 band
        return (i, 2 * w, [(off, off + 2 * w), (off + 2 * w, off + 4 * w)])

    # class A: w=25 bands 0..9, pairs
    for a in range(5):
        i0 = 2 * a
        supers.append(dict(
            kind="pair", f0=int(_STARTS[i0]), w=25,
            subs=[band_sub(i0 + j, 25, int(_CHOFF[i0 + j])) for j in range(2)],
        ))
    # class B: w=50 bands 10..21, pairs
    for a in range(6):
        i0 = 10 + 2 * a
        supers.append(dict(
            kind="pair", f0=int(_STARTS[i0]), w=50,
            subs=[band_sub(i0 + j, 50, int(_CHOFF[i0 + j])) for j in range(2)],
        ))
    # class C: w=100 bands 22..29, one super per band, subs = (c0, c1)
    for i in range(22, 30):
        off = int(_CHOFF[i])
        w = 100
        supers.append(dict(
            kind="cpair", f0=int(_STARTS[i]), w=w, nf=w,
            subs=[(i, w, [(off + r * 2 * w + c * w, off + r * 2 * w + (c + 1) * w)
                          for r in range(2)]) for c in range(2)],
        ))
    # class D: band 30 (w=399), supers per f-chunk, subs = (c0, c1)
    i = 30
    off = int(_CHOFF[i])
    w = 399
    for f0 in range(0, w, 128):
        f1 = min(f0 + 128, w)
        supers.append(dict(
            kind="cpair", f0=int(_STARTS[i]) + f0, w=w, nf=f1 - f0,
            subs=[(i, f1 - f0,
                   [(off + r * 2 * w + c * w + f0, off + r * 2 * w + c * w + f1)
                    for r in range(2)]) for c in range(2)],
        ))

    # weight slots: one per (super, sub, r), in traversal order
    slots = []
    for si, sup in enumerate(supers):
        for j, (band, K, chs) in enumerate(sup["subs"]):
            for r in range(2):
                slots.append(dict(super=si, sub=j, r=r, ch=chs[r]))

    # S2 strip columns: one ACT op per (super, band-different subs) or per super
    # (when both subs are the same band). Per-band ranges padded to EVEN width.
    s2ops = []  # (super_idx, sub_list, band, col)
    band_ncols = [0] * NBANDS
    per_super_ops = []
    for si, sup in enumerate(supers):
        bands = {b for (b, _, _) in sup["subs"]}
        if len(bands) == 1:
            per_super_ops.append((si, list(range(len(sup["subs"]))), sup["subs"][0][0]))
        else:
            for j, (band, K, _) in enumerate(sup["subs"]):
                per_super_ops.append((si, [j], band))
    band_cols = [[] for _ in range(NBANDS)]
    col = 0
    for band in range(NBANDS):
        ops_b = [(si, js) for (si, js, b) in per_super_ops if b == band]
        for k, (si, js) in enumerate(ops_b):
            s2ops.append((si, js, band, col + k))
        width = len(ops_b) + (len(ops_b) % 2)
        band_cols[band] = list(range(col, col + width))
        col += width
    return supers, slots, s2ops, band_cols, int(col)


SUPERS, SLOTS, S2OPS, BAND_S2COLS, N_S2COLS = _tables()
N_SLOTS = len(SLOTS)  # 92


def _pack_params(W, gamma, beta, bb):
    """Host-side preparation of the parameter-only tensors."""
    Wg = (W * gamma[None, :]).astype(np.float32)
    WgT = np.ascontiguousarray(Wg.T)
    wtp = np.zeros((128, N_SLOTS, 128), np.float32)
    for j, sl in enumerate(SLOTS):
        a, e = sl["ch"]
        wtp[: e - a, j, :] = WgT[a:e, :]
    uvb = np.zeros((128, 2, NBANDS), np.float32)
    for i in range(NBANDS):
        a, e = int(_CHOFF[i]), int(_CHOFF[i + 1])
        uvb[:, 0, i] = Wg[:, a:e].sum(axis=1)
        uvb[:, 1, i] = W[:, a:e] @ beta[a:e] + bb[i]
    return wtp, uvb


def _super_dmas(nc, x_d, sup, xt):
    """Issue the staging DMA(s) for one super tile.

    pair supers need one DMA per c (the (c, f) partition dim is not a single
    stride), cpair supers are a single 3D AP.
    """
    xr = x_d.bitcast(F32R)
    base = xr[0, 0, 0, 0]
    CS = F_TOT * T * 2          # c stride (elements)
    FS = T * 2                  # f stride
    off = sup["f0"] * FS
    if sup["kind"] == "pair":
        w = sup["w"]
        for c in range(2):
            ap = bass.AP(tensor=base.tensor, offset=base.offset + off + c * CS,
                         ap=[[FS, w], [FS * w, 2], [1, 1024]])
            nc.sync.dma_start(out=xt[c * w: (c + 1) * w, :, :], in_=ap)
    else:
        nf = sup["nf"]
        ap = bass.AP(tensor=base.tensor, offset=base.offset + off,
                     ap=[[FS, nf], [CS, 2], [1, 1024]])
        nc.sync.dma_start(out=xt[0:nf, :, :], in_=ap)


def _build_nc():
    nc = bacc.Bacc("TRN2")

    x_d = nc.dram_tensor("xb", [C_IN, F_TOT, T, 2], F32, kind="ExternalInput")
    wtp_d = nc.dram_tensor("wtp", [128, N_SLOTS, 128], F32, kind="ExternalInput")
    uvb_d = nc.dram_tensor("uvb", [128, 2, NBANDS], F32, kind="ExternalInput")
    y_d = nc.dram_tensor("y", [OUT_CH, NBANDS, T], F32, kind="ExternalOutput")

    # DRAM scratch for cross-partition broadcasts (written then read back)
    cvec_d = nc.dram_tensor("cvec_scratch", [1, NBANDS], F32)
    rpack_d = nc.dram_tensor("rpack_scratch", [NBANDS, 2], F32)

    with tile.TileContext(nc) as tc:
        with tc.tile_pool(name="persist", bufs=1) as persist, \
             tc.tile_pool(name="stagea", bufs=3) as stagea, \
             tc.tile_pool(name="stageb", bufs=3) as stageb, \
             tc.tile_pool(name="scratch", bufs=1) as scratchp, \
             tc.tile_pool(name="small", bufs=1) as small, \
             tc.tile_pool(name="psmain", bufs=6, space="PSUM") as psmain, \
             tc.tile_pool(name="psaux", bufs=1, space="PSUM") as psaux:

            # ------------- parameters (act-queue DMAs, in pieces) ----------
            piece = (N_SLOTS + N_WTP_PIECES - 1) // N_WTP_PIECES
            wtps = []
            slot_tile = []
            for p in range(N_WTP_PIECES):
                j0 = p * piece
                j1 = min(j0 + piece, N_SLOTS)
                wt = persist.tile([128, j1 - j0, 128], F32R, name=f"wtp{p}")
                nc.scalar.dma_start(out=wt, in_=wtp_d.bitcast(F32R)[:, j0:j1, :])
                wtps.append(wt)
                for j in range(j0, j1):
                    slot_tile.append((p, j - j0))
            uvb = persist.tile([128, 2, NBANDS], F32)
            nc.scalar.dma_start(out=uvb, in_=uvb_d[:])

            # one-hot matrix, M=32 slices: ohm[p, c] = (c == 31);
            # band b stationary = ohm[0:K, 31-b : 63-b] -> out row b
            ohm32 = small.tile([128, 63], F32)
            nc.vector.memset(ohm32, 0.0)
            nc.vector.memset(ohm32[:, 31:32], 1.0)
            ohm = small.tile([128, 63], F32R)
            nc.vector.tensor_copy(out=ohm, in_=ohm32)

            # per-band 1/(ch*T) constants -> [31, 1] via DRAM bounce
            cvec = small.tile([1, NBANDS], F32)
            for i, w in enumerate(WIDTHS):
                nc.vector.memset(cvec[0:1, i:i + 1], 1.0 / (4 * w * T))
            nc.sync.dma_start(out=cvec_d[:], in_=cvec)
            invn = small.tile([NBANDS, 1], F32)
            src = cvec_d[0:1, :]
            nc.sync.dma_start(
                out=invn,
                in_=bass.AP(tensor=src.tensor, offset=src.offset,
                            ap=[[1, NBANDS], [1, 1]]),
            )

            strip = small.tile([128, N_S2COLS], F32)
            nc.vector.memset(strip, 0.0)

            # ---------------- main streaming loop ------------------------
            s1ps = psaux.tile([32, T], F32)            # per-band sums, row = band
            osb = persist.tile([128, NBANDS, T], F32)  # raw y = Wg @ h

            band_nmm = {}
            for sl in SLOTS:
                b = SUPERS[sl["super"]]["subs"][sl["sub"]][0]
                band_nmm[b] = band_nmm.get(b, 0) + 1
            band_done = {b_: 0 for b_ in band_nmm}
            band_psum = {}

            xts = {}
            n_s1 = sum(2 * len(sup["subs"]) for sup in SUPERS)
            s1_idx = 0
            slot_iter = 0
            for si, sup in enumerate(SUPERS):
                nsub = len(sup["subs"])
                pool = stagea if si % 2 == 0 else stageb
                xt = pool.tile([128, nsub, 1024], F32R, tag=pool.name,
                               name=f"xt{si}")
                _super_dmas(nc, x_d, sup, xt)
                xts[si] = xt

                for j, (band, K, chs) in enumerate(sup["subs"]):
                    xv = xt[:, j, :].rearrange("p (t r) -> p t r", r=2)
                    if band not in band_psum:
                        band_psum[band] = psmain.tile([128, T], F32, tag="acc",
                                                      name=f"acc{band}")
                    for r in range(2):
                        pi, lj = slot_tile[slot_iter]
                        slot_iter += 1
                        band_done[band] += 1
                        nc.tensor.matmul(
                            band_psum[band][:],
                            wtps[pi][0:K, lj, :],
                            xv[0:K, :, r],
                            start=(band_done[band] == 1),
                            stop=(band_done[band] == band_nmm[band]),
                        )
                    for h in range(2):
                        s1_idx += 1
                        nc.tensor.matmul(
                            s1ps[:],
                            ohm[0:K, 31 - band: 63 - band],
                            xt[0:K, j, h * T: (h + 1) * T],
                            start=(s1_idx == 1),
                            stop=(s1_idx == n_s1),
                        )
                    if band_done[band] == band_nmm[band]:
                        acc = band_psum.pop(band)
                        nc.vector.tensor_copy(out=osb[:, band, :], in_=acc[:])

            # S2 ops (scalar engine square+accumulate), one per (super, band)
            # traversal aligned with supers for locality
            sq = scratchp.tile([128, 2048], F32)
            for (si, js, band, col) in sorted(S2OPS, key=lambda t: t[0]):
                sup = SUPERS[si]
                K = sup["subs"][js[0]][1]
                xt = xts[si]
                if len(js) == 1:
                    in_ap = xt.bitcast(F32)[0:K, js[0], :]
                    out_ap = sq[0:K, 0:1024]
                else:
                    in_ap = xt.bitcast(F32)[0:K, :, :]
                    out_ap = sq[0:K, 0: 1024 * len(js)]
                nc.scalar.activation(
                    out=out_ap, in_=in_ap, func=AFT.Square,
                    accum_out=strip[0:K, col: col + 1],
                )

            # ---------------- statistics -> r, rmu ------------------------
            strip_r = small.tile([128, N_S2COLS], F32R)
            nc.vector.tensor_copy(out=strip_r, in_=strip)
            s2ps = psaux.tile([32, N_S2COLS], F32)
            for band in range(NBANDS):
                cols = BAND_S2COLS[band]
                c0, c1 = cols[0], cols[-1] + 1
                nc.tensor.matmul(
                    s2ps[:, c0:c1],
                    ohm[0:128, 31 - band: 63 - band],
                    strip_r[:, c0:c1],
                    start=(band == 0), stop=(band == NBANDS - 1),
                )

            s1red = small.tile([NBANDS, 1], F32)
            nc.vector.tensor_reduce(out=s1red, in_=s1ps[0:NBANDS, :],
                                    axis=mybir.AxisListType.X, op=ALU.add)
            s2red = small.tile([NBANDS, 1], F32)
            nc.vector.tensor_reduce(out=s2red, in_=s2ps[0:NBANDS, :],
                                    axis=mybir.AxisListType.X, op=ALU.add)

            mu = small.tile([NBANDS, 1], F32)
            nc.vector.tensor_mul(out=mu, in0=s1red, in1=invn)
            ex2 = small.tile([NBANDS, 1], F32)
            nc.vector.tensor_mul(out=ex2, in0=s2red, in1=invn)
            musq = small.tile([NBANDS, 1], F32)
            nc.vector.tensor_mul(out=musq, in0=mu, in1=mu)
            var = small.tile([NBANDS, 1], F32)
            nc.vector.tensor_tensor(out=var, in0=ex2, in1=musq, op=ALU.subtract)
            epst = small.tile([NBANDS, 1], F32)
            nc.vector.memset(epst, EPS)
            std = small.tile([NBANDS, 1], F32)
            nc.scalar.activation(out=std, in_=var, func=AFT.Sqrt, bias=epst[:, 0:1])
            rpack = small.tile([NBANDS, 2], F32)
            nc.vector.reciprocal(out=rpack[:, 0:1], in_=std)
            nc.vector.tensor_mul(out=rpack[:, 1:2], in0=rpack[:, 0:1], in1=mu)

            # broadcast r/rmu to all partitions: one contiguous-per-partition DMA
            nc.sync.dma_start(out=rpack_d[:], in_=rpack)
            rbu = small.tile([128, NBANDS, 2], F32)
            src_r = rpack_d[0:1, 0:1]
            nc.sync.dma_start(
                out=rbu,
                in_=bass.AP(tensor=src_r.tensor, offset=src_r.offset,
                            ap=[[0, 128], [2, NBANDS], [1, 2]]),
            )

            # bias vectors: BB = (v + b) - rmu * u
            t_ru = small.tile([128, NBANDS], F32)
            nc.vector.tensor_mul(out=t_ru, in0=rbu[:, :, 1], in1=uvb[:, 0, :])
            bbv = small.tile([128, NBANDS], F32)
            nc.vector.tensor_tensor(out=bbv, in0=uvb[:, 1, :], in1=t_ru,
                                    op=ALU.subtract)

            # ------------- finalize in place + grouped stores --------------
            for (g0, g1) in OUT_GROUPS:
                for band in range(g0, g1):
                    nc.vector.tensor_scalar(
                        out=osb[:, band, :],
                        in0=osb[:, band, :],
                        scalar1=rbu[:, band, 0:1],
                        scalar2=bbv[:, band: band + 1],
                        op0=ALU.mult,
                        op1=ALU.add,
                    )
                nc.scalar.dma_start(out=y_d[:, g0:g1, :], in_=osb[:, g0:g1, :])

    nc.finalize()
    return nc


_NC_CACHE = None


def _get_nc():
    global _NC_CACHE
    if _NC_CACHE is None:
        _NC_CACHE = _build_nc()
    return _NC_CACHE


def kernel(x, gamma, beta, W, b):
    from concourse.bass_utils import run_bass_kernel_spmd

    x = np.asarray(x, dtype=np.float32)
    gamma = np.asarray(gamma, dtype=np.float32)
    beta = np.asarray(beta, dtype=np.float32)
    W = np.asarray(W, dtype=np.float32)
    b = np.asarray(b, dtype=np.float32)

    wtp, uvb = _pack_params(W, gamma, beta, b)
    nc = _get_nc()
    in_maps = [
        {"xb": np.ascontiguousarray(x[i]), "wtp": wtp, "uvb": uvb}
        for i in range(N_CORES)
    ]
    res = run_bass_kernel_spmd(nc, in_maps, list(range(N_CORES)))
    return np.stack([res.results[i]["y"] for i in range(N_CORES)], axis=0)
